# revision 19
# baseline (speedup 1.0000x reference)
"""NeRF emission-absorption raymarcher as a Bass/Tile kernel for 8 trn2 cores.

Sharding: batch axis B=8, one batch element per NeuronCore (embarrassingly
parallel over rays). Per core: 16384 rays x 128 samples.

Layout per core: rays on partitions (128 rays/subtile), samples along the free
dim, p-major ray order so every DMA moves large contiguous segments. One DVE
tensor_tensor_scan per chunk computes all subtiles' inclusive cumprods of
(1-d): state = xs*state + ind, where ind=1 at each subtile's leading column
resets the recurrence. Telescoping gives w[t] = T[t-1]-T[t] (one subtract) and
1-alpha = last scan column. Channel products (w*feat_c, w*len) land in a
[4, C, N+1] scratch whose last column carries the (1-alpha) correction, so a
single tensor_reduce(axis=X) emits the final [4, C] outputs per chunk. The
kernel is DMA-bound (~260us/pass vs ~115us pure-DMA roofline estimate).
"""

import sys

try:
    import concourse.bass as bass
except ImportError:  # fresh interpreter without the repo on sys.path
    sys.path.insert(0, "/opt/trn_rl_repo")
    import concourse.bass as bass

from contextlib import ExitStack, nullcontext

import numpy as np
import orjson

import concourse.tile as tile
import concourse.bass_utils as bass_utils
import concourse.bass2jax as bass2jax
from concourse import mybir
from concourse.bass_utils import run_bass_kernel_spmd


def _legalize_sync(bir_json: bytes) -> bytes:
    """Split multi-wait/multi-update sync_info into single-sync NoOp chains.

    This toolchain's walrus codegen encodes at most one sync wait and one
    sync update per TPB instruction ("Too many sync wait commands"), but the
    Tile framework emits instructions (and its final drain) carrying several.
    A NoOp on the same engine queue issued immediately before (waits) or
    after (updates) is semantically equivalent under in-order queue issue.
    """
    m = orjson.loads(bir_json)
    for fn in m["functions"]:
        for blk in fn["blocks"]:
            out = []
            for ins in blk["instructions"]:
                si = ins.get("sync_info")
                post = []
                if si:
                    waits = si.get("on_wait") or []
                    if len(waits) > 1:
                        for i, w in enumerate(waits[:-1]):
                            out.append(
                                {
                                    "debug": ins.get("debug", 0),
                                    "engine": ins["engine"],
                                    "ins": [],
                                    "outs": [],
                                    "name": f"{ins['name']}-lw{i}",
                                    "opcode": "NoOp",
                                    "text_hint": "legalize-wait",
                                    "sync_info": {"on_update": [], "on_wait": [w]},
                                }
                            )
                        si["on_wait"] = [waits[-1]]
                    ups = si.get("on_update") or []
                    if len(ups) > 1:
                        for i, u in enumerate(ups[1:]):
                            post.append(
                                {
                                    "debug": ins.get("debug", 0),
                                    "engine": ins["engine"],
                                    "ins": [],
                                    "outs": [],
                                    "name": f"{ins['name']}-lu{i}",
                                    "opcode": "NoOp",
                                    "text_hint": "legalize-update",
                                    "sync_info": {"on_update": [u], "on_wait": []},
                                }
                            )
                        si["on_update"] = ups[:1]
                out.append(ins)
                out.extend(post)
            blk["instructions"] = out
    return orjson.dumps(m)


_orig_compile_bir_kernel = bass_utils.compile_bir_kernel


def _compile_bir_kernel_legalized(bir_json, tmpdir, neff_name="file.neff"):
    return _orig_compile_bir_kernel(_legalize_sync(bir_json), tmpdir, neff_name)


bass_utils.compile_bir_kernel = _compile_bir_kernel_legalized
bass2jax.compile_bir_kernel = _compile_bir_kernel_legalized

P = 128            # partitions == rays per subtile
N = 128            # samples per ray
NP1 = N + 1
F = 3              # feature channels
B = 8              # batch (== number of cores)
R = 16384          # rays per batch element (per core)
C = 8              # subtiles per chunk (C*P rays per chunk)
NCHUNK = R // (P * C)
NCORES = 8

f32 = mybir.dt.float32
Alu = mybir.AluOpType
Act = mybir.ActivationFunctionType


def build(nchunk=NCHUNK, bufs=3, repeat=1, loop=None):
    nc = bass.Bass()
    rc = nchunk * C * P
    d_dram = nc.declare_dram_parameter("d", [rc, N], f32, isOutput=False)
    f_dram = nc.declare_dram_parameter("f", [rc, N * F], f32, isOutput=False)
    l_dram = nc.declare_dram_parameter("l", [rc, N], f32, isOutput=False)
    # out[p, ch, f4, t] with ray = p*(nchunk*C) + ch*C + t
    o_dram = nc.declare_dram_parameter("o", [P, nchunk * 4 * C], f32, isOutput=True)

    with ExitStack() as ctx:
        tc = ctx.enter_context(tile.TileContext(nc))
        cpool = ctx.enter_context(tc.tile_pool(name="const", bufs=1))
        opool = ctx.enter_context(tc.tile_pool(name="outp", bufs=1))
        pool = ctx.enter_context(tc.tile_pool(name="main", bufs=bufs))
        ind = cpool.tile([P, C * NP1], f32)
        # loop=K wraps the (statically addressed) body in a HW For_i so one
        # NEFF runs K identical passes — used only for wall-clock timing
        ctx.enter_context(tc.For_i(0, loop) if loop else nullcontext())
        # scan-reset indicator: 1.0 at each subtile's leading column
        # (idempotent, so re-running it per For_i pass is harmless)
        nc.vector.memset(ind[:], 0.0)
        nc.vector.memset(
            ind[:].rearrange("p (t x) -> p t x", x=NP1)[:, :, 0], 1.0
        )
        for _ in range(repeat):
            oa = opool.tile([P, nchunk * 4 * C], f32)
            oav = oa[:].rearrange("p (k f t) -> p k f t", f=4, t=C)
            for ch in range(nchunk):
                dx = pool.tile([P, C * N], f32)      # densities (dense)
                xs = pool.tile([P, C * NP1], f32)    # [0, 1-d] per subtile
                cpx = pool.tile([P, C * NP1], f32)   # reset-scan cumprod
                ft = pool.tile([P, C * N * F], f32)
                lt = pool.tile([P, C * N], f32)
                w = pool.tile([P, C * N], f32)
                pc = pool.tile([P, 4 * C * NP1], f32)  # products + oma columns

                # ray = p*q_total + q (p-major): each partition's C-subtile
                # slice is C contiguous DRAM rows => large contiguous segments
                nc.sync.dma_start(
                    out=dx[:].rearrange("p (t x) -> p t x", x=N),
                    in_=d_dram[:, :].rearrange("(p q) n -> p q n", p=P)[
                        :, ch * C : (ch + 1) * C, :
                    ],
                )
                nc.sync.dma_start(
                    out=ft[:].rearrange("p (t x) -> p t x", x=N * F),
                    in_=f_dram[:, :].rearrange("(p q) x -> p q x", p=P)[
                        :, ch * C : (ch + 1) * C, :
                    ],
                )
                nc.sync.dma_start(
                    out=lt[:].rearrange("p (t x) -> p t x", x=N),
                    in_=l_dram[:, :].rearrange("(p q) x -> p q x", p=P)[
                        :, ch * C : (ch + 1) * C, :
                    ],
                )

                xsv = xs[:].rearrange("p (t x) -> p t x", x=NP1)
                nc.vector.memset(xsv[:, :, 0], 0.0)
                # xs = 1 - d
                nc.scalar.activation(
                    xsv[:, :, 1:],
                    dx[:].rearrange("p (t x) -> p t x", x=N),
                    Act.Copy,
                    bias=1.0,
                    scale=-1.0,
                )
                # single cross-subtile scan: state = xs[t]*state + ind[t];
                # boundary (xs=0, ind=1) resets state to 1 => per-subtile
                # inclusive cumprod with leading 1. cpx[:,t,N] = 1 - alpha.
                nc.vector.tensor_tensor_scan(
                    out=cpx[:],
                    data0=xs[:],
                    data1=ind[:],
                    initial=0.0,
                    op0=Alu.mult,
                    op1=Alu.add,
                )

                cpxv = cpx[:].rearrange("p (t x) -> p t x", x=NP1)
                wv = w[:].rearrange("p (t n) -> p t n", n=N)
                ftv = ft[:].rearrange("p (t n c) -> p t n c", n=N, c=F)
                ltv = lt[:].rearrange("p (t n) -> p t n", n=N)
                pcv = pc[:].rearrange("p (f t x) -> p f t x", t=C, x=NP1)
                oma = cpxv[:, :, N]

                # telescoping: w[t] = T[t-1] - T[t] (= d[t] * absorption[t])
                nc.gpsimd.tensor_sub(wv, cpxv[:, :, 0:N], cpxv[:, :, 1:NP1])
                # channel products; col N carries the (1-alpha) correction so
                # one fused reduce yields the final outputs directly
                nc.vector.tensor_mul(pcv[:, 0, :, 0:N], wv, ftv[:, :, :, 0])
                nc.gpsimd.tensor_mul(pcv[:, 1, :, 0:N], wv, ftv[:, :, :, 1])
                nc.gpsimd.tensor_mul(pcv[:, 2, :, 0:N], wv, ftv[:, :, :, 2])
                nc.gpsimd.tensor_mul(pcv[:, 3, :, 0:N], wv, ltv)
                for c in range(F):
                    nc.scalar.activation(pcv[:, c, :, N], oma, Act.Copy)
                nc.gpsimd.tensor_mul(pcv[:, 3, :, N], oma, ltv[:, :, N - 1])
                # fused per-subtile reduction over the innermost axis:
                # [P, 4, C, NP1] -> [P, 4, C] written straight into the
                # pass-wide output accumulator
                nc.vector.tensor_reduce(
                    oav[:, ch], pcv, axis=mybir.AxisListType.X, op=Alu.add
                )
            nc.sync.dma_start(out=o_dram[:, :], in_=oa[:])
    return nc


_nc_cache = {}


def _get_nc():
    if "nc" not in _nc_cache:
        _nc_cache["nc"] = build()
    return _nc_cache["nc"]


def make_in_maps(rays_densities, rays_features, lengths):
    in_maps = []
    for b in range(B):
        in_maps.append(
            {
                "d": np.ascontiguousarray(
                    rays_densities[b, :, :, 0], dtype=np.float32
                ),
                "f": np.ascontiguousarray(rays_features[b], dtype=np.float32).reshape(
                    R, N * F
                ),
                "l": np.ascontiguousarray(lengths[b], dtype=np.float32),
            }
        )
    return in_maps


def unpack_out(o, nchunk=NCHUNK):
    # o: [P, nchunk, 4, C], ray = p*(nchunk*C) + ch*C + t
    return (
        np.asarray(o)
        .reshape(P, nchunk, 4, C)
        .transpose(0, 1, 3, 2)
        .reshape(nchunk * C * P, 4)
    )


def kernel(rays_densities, rays_features, lengths):
    assert rays_densities.shape == (B, R, N, 1)
    nc = _get_nc()
    in_maps = make_in_maps(rays_densities, rays_features, lengths)
    res = run_bass_kernel_spmd(nc, in_maps, list(range(NCORES))).results
    return np.stack([unpack_out(res[b]["o"]) for b in range(B)], axis=0)
